# revision 3
# baseline (speedup 1.0000x reference)
"""Distributed Bass kernel: single-head causal attention with column softmax.

Reference semantics (B=8, T=2048, D=1024, H=64):
    q = x @ Wq.T ; k = x @ Wk.T ; v = x @ Wv.T            # [b, t, h]
    scores = (q @ k.T) / sqrt(H)                           # [b, t, s]
    scores = where(tril, scores, -inf)                     # causal (t >= s)
    attn   = softmax(scores, axis=1)                       # over QUERY axis t
    out    = attn @ v                                      # [b, t, h]

(The reference's masked_fill(scores==0, -inf) quirk only affects the upper
triangle for these inputs -- verified numerically: no exact zeros below the
diagonal -- so it is equivalent to the causal mask.)

Sharding: pure data-parallel over batch, one batch per NeuronCore, no
collectives.  Per core we compute with scores TRANSPOSED (scoresT[s, t]) so
the axis-1 softmax reduction runs along the SBUF free axis, and fold the
1/Z[s] normalizer into v's rows so no extra pass over the TxT matrix is
needed.  Fully-masked 128x512 blocks are skipped in both the scores and the
output matmuls (triangular skipping).
"""

from contextlib import ExitStack

import numpy as np

B, T, D, H = 8, 2048, 1024, 64
P = 128                 # SBUF partitions / s-tile height
CH = 512                # free-dim chunk width (one PSUM bank of fp32)
NT = T // P             # 16 s-tiles
NCH = T // CH           # 4 t-chunks
ND = D // P             # 8 d-tiles
SCALE = 1.0 / float(np.sqrt(H))
N_CORES = 8
MM_DT = "float32r"      # tensor-engine matmul dtype: float32r (fast) or float32

_BUILT = {}


def _build_nc():
    """Build + compile the per-core Bass graph (identical on all 8 cores)."""
    import concourse.bass as bass
    import concourse.tile as tile
    from concourse import bacc, mybir
    from concourse.masks import make_identity

    f32 = mybir.dt.float32
    mmdt = getattr(mybir.dt, MM_DT)
    Exp = mybir.ActivationFunctionType.Exp
    ts = bass.ts


    nc = bacc.Bacc(None, target_bir_lowering=False, debug=False)

    xt_d = nc.declare_dram_parameter("xt", [D, T], mmdt, isOutput=False)
    wq_d = nc.declare_dram_parameter("wq", [D, H], mmdt, isOutput=False)
    wk_d = nc.declare_dram_parameter("wk", [D, H], mmdt, isOutput=False)
    wv_d = nc.declare_dram_parameter("wv", [D, H], mmdt, isOutput=False)
    out_d = nc.declare_dram_parameter("out", [H, T], f32, isOutput=True)

    with tile.TileContext(nc) as tc, ExitStack() as ctx:
        const = ctx.enter_context(tc.tile_pool(name="const", bufs=1))
        ident = const.tile([64, 64], f32)
        make_identity(nc, ident[:])
        zbias = const.tile([P, 1], f32)
        nc.gpsimd.memset(zbias[:], 0.0)

        stats = ctx.enter_context(tc.tile_pool(name="stats", bufs=1))
        zpart = stats.tile([P, NT, NCH], f32)   # per (s-tile, chunk) partial sums
        zsum = stats.tile([P, NT], f32)         # Z per s row
        zrec = stats.tile([P, NT], f32)         # 1/Z
        nc.vector.memset(zpart[:], 0.0)

        xt_pool = ctx.enter_context(tc.tile_pool(name="xt", bufs=ND))
        w_pool = ctx.enter_context(tc.tile_pool(name="w", bufs=3))
        qkv_pool = ctx.enter_context(tc.tile_pool(name="qkv", bufs=3))
        v_pool = ctx.enter_context(tc.tile_pool(name="v", bufs=2))
        exp_pool = ctx.enter_context(tc.tile_pool(name="expT", bufs=4))
        out_pool = ctx.enter_context(tc.tile_pool(name="outsb", bufs=1))

        # ---- input DMAs -------------------------------------------------
        xts = []
        for n in range(ND):
            t = xt_pool.tile([P, T], mmdt, tag="xt")
            nc.sync.dma_start(t[:], xt_d[ts(n, P), :])
            xts.append(t)

        w_sb = {}
        for name, wd in (("q", wq_d), ("k", wk_d), ("v", wv_d)):
            w = w_pool.tile([P, ND, H], mmdt, tag="w")
            nc.sync.dma_start(w[:], wd.rearrange("(n p) h -> p n h", p=P))
            w_sb[name] = w

        # ---- projections: qT/kT/vT = [64, T] (h on partitions) ----------
        qT = qkv_pool.tile([64, T], mmdt, tag="qkv")
        kT = qkv_pool.tile([64, T], mmdt, tag="qkv")
        vT = qkv_pool.tile([64, T], f32, tag="qkv")
        v_sb = v_pool.tile([P, NT, H], f32, tag="v")    # v in [s, h] layout
        vp_sb = v_pool.tile([P, NT, H], mmdt, tag="v")   # v / Z[s]

        with tc.tile_pool(name="proj_ps", bufs=2, space="PSUM") as proj_ps, \
             tc.tile_pool(name="tr_ps", bufs=2, space="PSUM") as tr_ps:
            for name, dest in (("q", qT), ("k", kT), ("v", vT)):
                w = w_sb[name]
                for c in range(NCH):
                    ps = proj_ps.tile([64, CH], f32, tag="proj")
                    for n in range(ND):
                        nc.tensor.matmul(
                            ps[:],
                            w[:, n, :],
                            xts[n][:, ts(c, CH)],
                            start=(n == 0),
                            stop=(n == ND - 1),
                        )
                    nc.scalar.copy(dest[:, ts(c, CH)], ps[:])

            # vT -> v ([s, h] layout) via PE transpose
            for i in range(NT):
                tp = tr_ps.tile([P, H], f32, tag="tr")
                nc.tensor.transpose(tp[:], vT[:, ts(i, P)], ident[:])
                nc.vector.tensor_copy(v_sb[:, i, :], tp[:])

        # ---- scores + column softmax + output, s-tile pipelined ---------
        with tc.tile_pool(name="sc_ps", bufs=3, space="PSUM") as sc_ps, \
             tc.tile_pool(name="out_ps", bufs=4, space="PSUM") as out_ps:
            outp = [out_ps.tile([64, CH], f32, tag="op", name=f"outp{j}")
                    for j in range(NCH)]
            outsb = out_pool.tile([64, T], f32)
            expts = [None] * NT

            def scores_block(i):
                j0 = i // 4
                et = exp_pool.tile([P, T], mmdt, tag="expT")
                expts[i] = et
                for j in range(j0, NCH):
                    sc = sc_ps.tile([P, CH], f32, tag="sc")
                    nc.tensor.matmul(
                        sc[:],
                        kT[:, ts(i, P)],
                        qT[:, ts(j, CH)],
                        start=True,
                        stop=True,
                    )
                    if j == j0:
                        # diagonal-straddling chunk: exp, zero the invalid
                        # (t < s) corner, then row-sum the survivors
                        nc.scalar.activation(
                            et[:, ts(j, CH)], sc[:], Exp,
                            bias=zbias[:], scale=SCALE,
                        )
                        nc.gpsimd.affine_select(
                            out=et[:, ts(j, CH)],
                            in_=et[:, ts(j, CH)],
                            compare_op=mybir.AluOpType.is_ge,
                            fill=0.0,
                            base=CH * j - P * i,
                            channel_multiplier=-1,
                            pattern=[[1, CH]],
                        )
                        nc.vector.tensor_reduce(
                            zpart[:, i, 0:1], et[:, ts(j, CH)],
                            axis=mybir.AxisListType.X,
                            op=mybir.AluOpType.add,
                        )
                    else:
                        nc.scalar.activation(
                            et[:, ts(j, CH)], sc[:], Exp,
                            bias=zbias[:], scale=SCALE,
                            accum_out=zpart[:, i, j - j0:j - j0 + 1],
                        )
                # Z, 1/Z, v' = v * (1/Z) for this s-tile
                nc.vector.tensor_reduce(
                    zsum[:, i:i + 1], zpart[:, i, :],
                    axis=mybir.AxisListType.X, op=mybir.AluOpType.add,
                )
                nc.vector.reciprocal(zrec[:, i:i + 1], zsum[:, i:i + 1])
                nc.vector.tensor_scalar_mul(
                    vp_sb[:, i, :], v_sb[:, i, :], zrec[:, i:i + 1]
                )

            def out_block(i):
                j0 = i // 4
                et = expts[i]
                for j in range(j0, NCH):
                    nc.tensor.matmul(
                        outp[j][:],
                        vp_sb[:, i, :],
                        et[:, ts(j, CH)],
                        start=(i == 0),
                        stop=(i == 4 * j + 3),
                    )
                    if i == 4 * j + 3:
                        nc.scalar.copy(outsb[:, ts(j, CH)], outp[j][:])
                expts[i] = None

            SKEW = 2
            for i in range(NT):
                scores_block(i)
                if i >= SKEW:
                    out_block(i - SKEW)
            for i in range(NT - SKEW, NT):
                out_block(i)

            nc.sync.dma_start(out_d[:], outsb[:])

    nc.compile()
    return nc


def _get_nc():
    if "nc" not in _BUILT:
        _BUILT["nc"] = _build_nc()
    return _BUILT["nc"]


def _make_in_maps(x, Wk, Wq, Wv):
    x = np.ascontiguousarray(np.asarray(x, dtype=np.float32))
    wq_t = np.ascontiguousarray(np.asarray(Wq, np.float32).T)
    wk_t = np.ascontiguousarray(np.asarray(Wk, np.float32).T)
    wv_t = np.ascontiguousarray(np.asarray(Wv, np.float32).T)
    in_maps = []
    for b in range(N_CORES):
        in_maps.append({
            "xt": np.ascontiguousarray(x[b].T),
            "wq": wq_t,
            "wk": wk_t,
            "wv": wv_t,
        })
    return in_maps


def _run(x, Wk, Wq, Wv, **run_kwargs):
    from concourse.bass_utils import run_bass_kernel_spmd

    nc = _get_nc()
    in_maps = _make_in_maps(x, Wk, Wq, Wv)
    res = run_bass_kernel_spmd(nc, in_maps, core_ids=list(range(N_CORES)),
                               **run_kwargs)
    out = np.stack([np.asarray(res.results[b]["out"]).T
                    for b in range(N_CORES)]).astype(np.float32)
    return out, res


def kernel(x, Wk, Wq, Wv):
    out, _ = _run(x, Wk, Wq, Wv)
    return out


# revision 8
# speedup vs baseline: 1.1285x; 1.1285x over previous
"""Distributed Bass kernel: single-head causal attention with column softmax.

Reference semantics (B=8, T=2048, D=1024, H=64):
    q = x @ Wq.T ; k = x @ Wk.T ; v = x @ Wv.T            # [b, t, h]
    scores = (q @ k.T) / sqrt(H)                           # [b, t, s]
    scores = where(tril, scores, -inf)                     # causal (t >= s)
    attn   = softmax(scores, axis=1)                       # over QUERY axis t
    out    = attn @ v                                      # [b, t, h]

(The reference's masked_fill(scores==0, -inf) quirk only affects the upper
triangle for these inputs -- verified numerically: no exact zeros below the
diagonal -- so it is equivalent to the causal mask.)

Sharding: pure data-parallel over batch, one batch per NeuronCore, no
collectives.  Per core we compute with scores TRANSPOSED (scoresT[s, t]) so
the axis-1 softmax reduction runs along the SBUF free axis, and fold the
1/Z[s] normalizer into v's rows so no extra pass over the TxT matrix is
needed.  Fully-masked 128x512 blocks are skipped in both the scores and the
output matmuls (triangular skipping).
"""

from contextlib import ExitStack

import numpy as np

B, T, D, H = 8, 2048, 1024, 64
P = 128                 # SBUF partitions / s-tile height
CH = 512                # free-dim chunk width (one PSUM bank of fp32)
NT = T // P             # 16 s-tiles
NCH = T // CH           # 4 t-chunks
ND = D // P             # 8 d-tiles
SCALE = 1.0 / float(np.sqrt(H))
N_CORES = 8
MM_DT = "float32r"      # tensor-engine matmul dtype: float32r (fast) or float32

_BUILT = {}


def _build_nc():
    """Build + compile the per-core Bass graph (identical on all 8 cores)."""
    import concourse.bass as bass
    import concourse.tile as tile
    from concourse import bacc, mybir
    from concourse.masks import make_identity

    f32 = mybir.dt.float32
    mmdt = getattr(mybir.dt, MM_DT)
    Exp = mybir.ActivationFunctionType.Exp
    ts = bass.ts


    nc = bacc.Bacc(None, target_bir_lowering=False, debug=False)

    xt_d = nc.declare_dram_parameter("xt", [D, T], mmdt, isOutput=False)
    wq_d = nc.declare_dram_parameter("wq", [D, H], mmdt, isOutput=False)
    wk_d = nc.declare_dram_parameter("wk", [D, H], mmdt, isOutput=False)
    wv_d = nc.declare_dram_parameter("wv", [D, H], mmdt, isOutput=False)
    out_d = nc.declare_dram_parameter("out", [H, T], f32, isOutput=True)

    with tile.TileContext(nc) as tc, ExitStack() as ctx:
        const = ctx.enter_context(tc.tile_pool(name="const", bufs=1))
        ident = const.tile([64, 64], f32)
        make_identity(nc, ident[:])
        zbias = const.tile([P, 1], f32)
        nc.gpsimd.memset(zbias[:], 0.0)

        stats = ctx.enter_context(tc.tile_pool(name="stats", bufs=1))
        zpart = stats.tile([P, NT, 2], f32)     # per (s-tile, pair) partial sums
        zsum = stats.tile([P, NT], f32)         # Z per s row
        zrec = stats.tile([P, NT], f32)         # 1/Z
        nc.vector.memset(zpart[:], 0.0)

        xt_pool = ctx.enter_context(tc.tile_pool(name="xt", bufs=ND))
        w_pool = ctx.enter_context(tc.tile_pool(name="w", bufs=3))
        qkv_pool = ctx.enter_context(tc.tile_pool(name="qkv", bufs=3))
        v_pool = ctx.enter_context(tc.tile_pool(name="v", bufs=2))
        exp_pool = ctx.enter_context(tc.tile_pool(name="expT", bufs=4))
        out_pool = ctx.enter_context(tc.tile_pool(name="outsb", bufs=1))

        # ---- input DMAs (weights first so projections can start on the
        # first xt tile instead of after the whole 8 MB stream) ------------
        w_sb = {}
        for name, wd in (("q", wq_d), ("k", wk_d), ("v", wv_d)):
            w = w_pool.tile([P, ND, H], mmdt, tag="w")
            nc.sync.dma_start(w[:], wd.rearrange("(n p) h -> p n h", p=P))
            w_sb[name] = w

        xts = []
        for n in range(ND):
            t = xt_pool.tile([P, T], mmdt, tag="xt")
            nc.sync.dma_start(t[:], xt_d[ts(n, P), :])
            xts.append(t)

        # ---- projections: qT/kT/vT = [64, T] (h on partitions) ----------
        qT = qkv_pool.tile([64, T], mmdt, tag="qkv")
        kT = qkv_pool.tile([64, T], mmdt, tag="qkv")
        vT = qkv_pool.tile([64, T], f32, tag="qkv")
        v_sb = v_pool.tile([P, NT, H], f32, tag="v")    # v in [s, h] layout
        vp_sb = v_pool.tile([P, NT, H], mmdt, tag="v")   # v / Z[s]

        # q and k accumulate across d-tiles as each xt tile lands (8 live
        # PSUM banks); v runs after, overlapping the start of scores.
        with tc.tile_pool(name="proj_ps", bufs=8, space="PSUM") as proj_ps:
            qk_ps = {(name, c): proj_ps.tile([64, CH], f32, tag="proj",
                                             name=f"ps_{name}{c}")
                     for name in ("q", "k") for c in range(NCH)}
            for n in range(ND):
                for name in ("q", "k"):
                    for c in range(NCH):
                        nc.tensor.matmul(
                            qk_ps[(name, c)][:],
                            w_sb[name][:, n, :],
                            xts[n][:, ts(c, CH)],
                            start=(n == 0),
                            stop=(n == ND - 1),
                        )
            for c in range(NCH):
                nc.scalar.copy(qT[:, ts(c, CH)], qk_ps[("q", c)][:])
                nc.vector.tensor_copy(kT[:, ts(c, CH)], qk_ps[("k", c)][:])

        with tc.tile_pool(name="v_ps", bufs=4, space="PSUM") as v_ps, \
             tc.tile_pool(name="tr_ps", bufs=2, space="PSUM") as tr_ps:
            for c in range(NCH):
                ps = v_ps.tile([64, CH], f32, tag="vproj")
                for n in range(ND):
                    nc.tensor.matmul(
                        ps[:],
                        w_sb["v"][:, n, :],
                        xts[n][:, ts(c, CH)],
                        start=(n == 0),
                        stop=(n == ND - 1),
                    )
                nc.scalar.copy(vT[:, ts(c, CH)], ps[:])

            # vT -> v ([s, h] layout) via PE transpose
            for i in range(NT):
                tp = tr_ps.tile([P, H], f32, tag="tr")
                nc.tensor.transpose(tp[:], vT[:, ts(i, P)], ident[:])
                nc.vector.tensor_copy(v_sb[:, i, :], tp[:])

        # ---- scores + column softmax + output, s-tile pipelined ---------
        with tc.tile_pool(name="sc_ps", bufs=2, space="PSUM") as sc_ps, \
             tc.tile_pool(name="out_ps", bufs=4, space="PSUM") as out_ps:
            outp = [out_ps.tile([64, CH], f32, tag="op", name=f"outp{j}")
                    for j in range(NCH)]
            outsb = out_pool.tile([64, T], f32)
            expts = [None] * NT

            def scores_block(i):
                j0 = i // 4
                et = exp_pool.tile([P, T], mmdt, tag="expT")
                expts[i] = et
                # chunks j0..3, processed as pairs so exp runs 1024 wide
                pairs = [(j0, min(j0 + 1, NCH - 1) if j0 + 1 < NCH else None)]
                if j0 + 2 < NCH:
                    pairs.append((j0 + 2, j0 + 3 if j0 + 3 < NCH else None))
                for pi, (ja, jb) in enumerate(pairs):
                    pw = CH if jb is None else 2 * CH
                    sc = sc_ps.tile([P, 2 * CH], f32, tag="sc")
                    for h, j in enumerate([ja] if jb is None else [ja, jb]):
                        nc.tensor.matmul(
                            sc[:, ts(h, CH)],
                            kT[:, ts(i, P)],
                            qT[:, ts(j, CH)],
                            start=True,
                            stop=True,
                        )
                    erange = et[:, CH * ja:CH * ja + pw]
                    if pi == 0:
                        # first pair holds the diagonal: exp everything,
                        # zero the invalid (t < s) corner, then row-sum
                        nc.scalar.activation(
                            erange, sc[:, :pw], Exp,
                            bias=zbias[:], scale=SCALE,
                        )
                        w = P * i - CH * j0 + P   # dead prefix + diag block
                        nc.gpsimd.affine_select(
                            out=et[:, CH * j0:CH * j0 + w],
                            in_=et[:, CH * j0:CH * j0 + w],
                            compare_op=mybir.AluOpType.is_ge,
                            fill=0.0,
                            base=CH * j0 - P * i,
                            channel_multiplier=-1,
                            pattern=[[1, w]],
                        )
                        nc.vector.tensor_reduce(
                            zpart[:, i, 0:1], erange,
                            axis=mybir.AxisListType.X,
                            op=mybir.AluOpType.add,
                        )
                    else:
                        nc.scalar.activation(
                            erange, sc[:, :pw], Exp,
                            bias=zbias[:], scale=SCALE,
                            accum_out=zpart[:, i, 1:2],
                        )
                # Z, 1/Z, v' = v * (1/Z) for this s-tile
                nc.vector.tensor_reduce(
                    zsum[:, i:i + 1], zpart[:, i, :],
                    axis=mybir.AxisListType.X, op=mybir.AluOpType.add,
                )
                nc.vector.reciprocal(zrec[:, i:i + 1], zsum[:, i:i + 1])
                nc.vector.tensor_scalar_mul(
                    vp_sb[:, i, :], v_sb[:, i, :], zrec[:, i:i + 1]
                )

            def out_block(i):
                j0 = i // 4
                et = expts[i]
                for j in range(j0, NCH):
                    nc.tensor.matmul(
                        outp[j][:],
                        vp_sb[:, i, :],
                        et[:, ts(j, CH)],
                        start=(i == 0),
                        stop=(i == 4 * j + 3),
                    )
                    if i == 4 * j + 3:
                        nc.vector.tensor_copy(outsb[:, ts(j, CH)], outp[j][:])
                        nc.sync.dma_start(out_d[:, ts(j, CH)],
                                          outsb[:, ts(j, CH)])
                expts[i] = None

            SKEW = 2
            for i in range(NT):
                scores_block(i)
                if i >= SKEW:
                    out_block(i - SKEW)
            for i in range(NT - SKEW, NT):
                out_block(i)

    nc.compile()
    return nc


def _get_nc():
    if "nc" not in _BUILT:
        _BUILT["nc"] = _build_nc()
    return _BUILT["nc"]


def _make_in_maps(x, Wk, Wq, Wv):
    x = np.ascontiguousarray(np.asarray(x, dtype=np.float32))
    wq_t = np.ascontiguousarray(np.asarray(Wq, np.float32).T)
    wk_t = np.ascontiguousarray(np.asarray(Wk, np.float32).T)
    wv_t = np.ascontiguousarray(np.asarray(Wv, np.float32).T)
    in_maps = []
    for b in range(N_CORES):
        in_maps.append({
            "xt": np.ascontiguousarray(x[b].T),
            "wq": wq_t,
            "wk": wk_t,
            "wv": wv_t,
        })
    return in_maps


def _run(x, Wk, Wq, Wv, **run_kwargs):
    from concourse.bass_utils import run_bass_kernel_spmd

    nc = _get_nc()
    in_maps = _make_in_maps(x, Wk, Wq, Wv)
    res = run_bass_kernel_spmd(nc, in_maps, core_ids=list(range(N_CORES)),
                               **run_kwargs)
    out = np.stack([np.asarray(res.results[b]["out"]).T
                    for b in range(N_CORES)]).astype(np.float32)
    return out, res


def kernel(x, Wk, Wq, Wv):
    out, _ = _run(x, Wk, Wq, Wv)
    return out


# revision 10
# speedup vs baseline: 1.2313x; 1.0910x over previous
"""Distributed Bass kernel: single-head causal attention with column softmax.

Reference semantics (B=8, T=2048, D=1024, H=64):
    q = x @ Wq.T ; k = x @ Wk.T ; v = x @ Wv.T            # [b, t, h]
    scores = (q @ k.T) / sqrt(H)                           # [b, t, s]
    scores = where(tril, scores, -inf)                     # causal (t >= s)
    attn   = softmax(scores, axis=1)                       # over QUERY axis t
    out    = attn @ v                                      # [b, t, h]

(The reference's masked_fill(scores==0, -inf) quirk only affects the upper
triangle for these inputs -- verified numerically: no exact zeros below the
diagonal -- so it is equivalent to the causal mask.)

Sharding: pure data-parallel over batch, one batch per NeuronCore, no
collectives.  Per core we compute with scores TRANSPOSED (scoresT[s, t]) so
the axis-1 softmax reduction runs along the SBUF free axis, and fold the
1/Z[s] normalizer into v's rows so no extra pass over the TxT matrix is
needed.  Fully-masked 128x512 blocks are skipped in both the scores and the
output matmuls (triangular skipping).
"""

from contextlib import ExitStack

import numpy as np

B, T, D, H = 8, 2048, 1024, 64
P = 128                 # SBUF partitions / s-tile height
CH = 512                # free-dim chunk width (one PSUM bank of fp32)
NT = T // P             # 16 s-tiles
NCH = T // CH           # 4 t-chunks
ND = D // P             # 8 d-tiles
SCALE = 1.0 / float(np.sqrt(H))
N_CORES = 8
MM_DT = "float32r"      # projection matmul dtype: float32r (fast) or float32
PC_DT = "bfloat16"      # phase-C operand dtype (kT/qT/expT/vp): bfloat16 or MM_DT

_BUILT = {}


def _build_nc():
    """Build + compile the per-core Bass graph (identical on all 8 cores)."""
    import concourse.bass as bass
    import concourse.tile as tile
    from concourse import bacc, mybir
    from concourse.masks import make_identity

    f32 = mybir.dt.float32
    mmdt = getattr(mybir.dt, MM_DT)
    pcdt = getattr(mybir.dt, PC_DT)
    Exp = mybir.ActivationFunctionType.Exp
    ts = bass.ts


    nc = bacc.Bacc(None, target_bir_lowering=False, debug=False)

    xt_d = nc.declare_dram_parameter("xt", [D, T], mmdt, isOutput=False)
    wq_d = nc.declare_dram_parameter("wq", [D, H], mmdt, isOutput=False)
    wk_d = nc.declare_dram_parameter("wk", [D, H], mmdt, isOutput=False)
    wv_d = nc.declare_dram_parameter("wv", [D, H], mmdt, isOutput=False)
    out_d = nc.declare_dram_parameter("out", [H, T], f32, isOutput=True)

    with tile.TileContext(nc) as tc, ExitStack() as ctx:
        const = ctx.enter_context(tc.tile_pool(name="const", bufs=1))
        ident = const.tile([64, 64], f32)
        make_identity(nc, ident[:])
        zbias = const.tile([P, 1], f32)
        nc.gpsimd.memset(zbias[:], 0.0)
        iotap = const.tile([P, 1], f32)
        nc.gpsimd.iota(iotap[:], pattern=[[0, 1]], base=0,
                       channel_multiplier=1,
                       allow_small_or_imprecise_dtypes=True)
        col128 = const.tile([P, 1], f32)
        nc.gpsimd.memset(col128[:], float(P))

        stats = ctx.enter_context(tc.tile_pool(name="stats", bufs=1))
        zpart = stats.tile([P, NT, 2], f32)     # per (s-tile, pair) partial sums
        zsum = stats.tile([P, NT], f32)         # Z per s row
        zrec = stats.tile([P, NT], f32)         # 1/Z
        nc.vector.memset(zpart[:], 0.0)

        xt_pool = ctx.enter_context(tc.tile_pool(name="xt", bufs=ND))
        w_pool = ctx.enter_context(tc.tile_pool(name="w", bufs=3))
        qkv_pool = ctx.enter_context(tc.tile_pool(name="qkv", bufs=3))
        v_pool = ctx.enter_context(tc.tile_pool(name="v", bufs=2))
        exp_pool = ctx.enter_context(tc.tile_pool(name="expT", bufs=5))
        out_pool = ctx.enter_context(tc.tile_pool(name="outsb", bufs=1))

        # ---- input DMAs (weights first so projections can start on the
        # first xt tile instead of after the whole 8 MB stream) ------------
        w_sb = {}
        for name, wd in (("q", wq_d), ("k", wk_d), ("v", wv_d)):
            w = w_pool.tile([P, ND, H], mmdt, tag="w")
            nc.sync.dma_start(w[:], wd.rearrange("(n p) h -> p n h", p=P))
            w_sb[name] = w

        xts = []
        for n in range(ND):
            t = xt_pool.tile([P, T], mmdt, tag="xt")
            nc.sync.dma_start(t[:], xt_d[ts(n, P), :])
            xts.append(t)

        # ---- projections: qT/kT/vT = [64, T] (h on partitions) ----------
        qT = qkv_pool.tile([64, T], pcdt, tag="qkv")
        kT = qkv_pool.tile([64, T], pcdt, tag="qkv")
        vT = qkv_pool.tile([64, T], f32, tag="qkv")
        v_sb = v_pool.tile([P, NT, H], f32, tag="v")    # v in [s, h] layout
        vp_sb = v_pool.tile([P, NT, H], pcdt, tag="v")   # v / Z[s]

        # q and k accumulate across d-tiles as each xt tile lands (8 live
        # PSUM banks); v runs after, overlapping the start of scores.
        with tc.tile_pool(name="proj_ps", bufs=8, space="PSUM") as proj_ps:
            qk_ps = {(name, c): proj_ps.tile([64, CH], f32, tag="proj",
                                             name=f"ps_{name}{c}")
                     for name in ("q", "k") for c in range(NCH)}
            for n in range(ND):
                for name in ("q", "k"):
                    for c in range(NCH):
                        nc.tensor.matmul(
                            qk_ps[(name, c)][:],
                            w_sb[name][:, n, :],
                            xts[n][:, ts(c, CH)],
                            start=(n == 0),
                            stop=(n == ND - 1),
                        )
            for c in range(NCH):
                nc.scalar.copy(qT[:, ts(c, CH)], qk_ps[("q", c)][:])
                nc.vector.tensor_copy(kT[:, ts(c, CH)], qk_ps[("k", c)][:])

        with tc.tile_pool(name="v_ps", bufs=4, space="PSUM") as v_ps, \
             tc.tile_pool(name="tr_ps", bufs=2, space="PSUM") as tr_ps:
            for c in range(NCH):
                ps = v_ps.tile([64, CH], f32, tag="vproj")
                for n in range(ND):
                    nc.tensor.matmul(
                        ps[:],
                        w_sb["v"][:, n, :],
                        xts[n][:, ts(c, CH)],
                        start=(n == 0),
                        stop=(n == ND - 1),
                    )
                nc.scalar.copy(vT[:, ts(c, CH)], ps[:])

            # vT -> v ([s, h] layout) via PE transpose
            for i in range(NT):
                tp = tr_ps.tile([P, H], f32, tag="tr")
                nc.tensor.transpose(tp[:], vT[:, ts(i, P)], ident[:])
                nc.vector.tensor_copy(v_sb[:, i, :], tp[:])

        # ---- scores + column softmax + output, s-tile pipelined ---------
        with tc.tile_pool(name="sc_ps", bufs=2, space="PSUM") as sc_ps, \
             tc.tile_pool(name="out_ps", bufs=4, space="PSUM") as out_ps:
            outp = [out_ps.tile([64, CH], f32, tag="op", name=f"outp{j}")
                    for j in range(NCH)]
            outsb = out_pool.tile([64, T], f32)
            expts = [None] * NT

            def scores_block(i):
                j0 = i // 4
                off = P * i - CH * j0           # dead-prefix width in chunk j0
                et = exp_pool.tile([P, T], pcdt, tag="expT")
                expts[i] = et
                # chunks j0..3, processed as pairs so exp runs up to 1024 wide
                pairs = [(j0, j0 + 1 if j0 + 1 < NCH else None)]
                if j0 + 2 < NCH:
                    pairs.append((j0 + 2, j0 + 3 if j0 + 3 < NCH else None))
                for pi, (ja, jb) in enumerate(pairs):
                    pw = CH if jb is None else 2 * CH
                    sc = sc_ps.tile([P, 2 * CH], f32, tag="sc")
                    for h, j in enumerate([ja] if jb is None else [ja, jb]):
                        nc.tensor.matmul(
                            sc[:, ts(h, CH)],
                            kT[:, ts(i, P)],
                            qT[:, ts(j, CH)],
                            start=True,
                            stop=True,
                        )
                    if pi == 0:
                        # first pair holds the diagonal.  Columns before the
                        # diagonal block (t < 128*i) are fully masked: skip
                        # them entirely.  Pre-mask the in-block triangle to
                        # -FLT_MAX in PSUM (row p keeps cols >= p), so exp
                        # maps it to 0 and the ACT accumulator sums only the
                        # valid region.
                        nc.vector.tensor_mask_reduce(
                            out=sc[:, off:off + P],
                            in_=sc[:, off:off + P],
                            mask_start=iotap[:],
                            mask_end=col128[:],
                            scale=1.0,
                            accum_in=0.0,
                            op=mybir.AluOpType.add,
                        )
                        nc.scalar.activation(
                            et[:, P * i:CH * ja + pw], sc[:, off:pw], Exp,
                            bias=zbias[:], scale=SCALE,
                            accum_out=zpart[:, i, 0:1],
                        )
                    else:
                        nc.scalar.activation(
                            et[:, CH * ja:CH * ja + pw], sc[:, :pw], Exp,
                            bias=zbias[:], scale=SCALE,
                            accum_out=zpart[:, i, 1:2],
                        )
                # Z, 1/Z, v' = v * (1/Z) for this s-tile
                nc.vector.tensor_reduce(
                    zsum[:, i:i + 1], zpart[:, i, :],
                    axis=mybir.AxisListType.X, op=mybir.AluOpType.add,
                )
                nc.vector.reciprocal(zrec[:, i:i + 1], zsum[:, i:i + 1])
                nc.vector.tensor_scalar_mul(
                    vp_sb[:, i, :], v_sb[:, i, :], zrec[:, i:i + 1]
                )

            def out_block(i):
                j0 = i // 4
                et = expts[i]
                for j in range(j0, NCH):
                    if j == j0:
                        o = P * i - CH * j0     # skip the never-written prefix
                        rhs = et[:, P * i:CH * (j + 1)]
                        dst = outp[j][:, o:CH]
                    else:
                        rhs = et[:, ts(j, CH)]
                        dst = outp[j][:]
                    nc.tensor.matmul(
                        dst,
                        vp_sb[:, i, :],
                        rhs,
                        start=(i == 0),
                        stop=(i == 4 * j + 3),
                    )
                    if i == 4 * j + 3:
                        nc.vector.tensor_copy(outsb[:, ts(j, CH)], outp[j][:])
                        nc.sync.dma_start(out_d[:, ts(j, CH)],
                                          outsb[:, ts(j, CH)])
                expts[i] = None

            # batch s-tiles in pairs for denser PE bursts (HAM warmth)
            GROUP = 2
            for g in range(0, NT, GROUP):
                for i in range(g, g + GROUP):
                    scores_block(i)
                if g >= GROUP:
                    for i in range(g - GROUP, g):
                        out_block(i)
            for i in range(NT - GROUP, NT):
                out_block(i)

    nc.compile()
    return nc


def _get_nc():
    if "nc" not in _BUILT:
        _BUILT["nc"] = _build_nc()
    return _BUILT["nc"]


def _make_in_maps(x, Wk, Wq, Wv):
    x = np.ascontiguousarray(np.asarray(x, dtype=np.float32))
    wq_t = np.ascontiguousarray(np.asarray(Wq, np.float32).T)
    wk_t = np.ascontiguousarray(np.asarray(Wk, np.float32).T)
    wv_t = np.ascontiguousarray(np.asarray(Wv, np.float32).T)
    in_maps = []
    for b in range(N_CORES):
        in_maps.append({
            "xt": np.ascontiguousarray(x[b].T),
            "wq": wq_t,
            "wk": wk_t,
            "wv": wv_t,
        })
    return in_maps


def _run(x, Wk, Wq, Wv, **run_kwargs):
    from concourse.bass_utils import run_bass_kernel_spmd

    nc = _get_nc()
    in_maps = _make_in_maps(x, Wk, Wq, Wv)
    res = run_bass_kernel_spmd(nc, in_maps, core_ids=list(range(N_CORES)),
                               **run_kwargs)
    out = np.stack([np.asarray(res.results[b]["out"]).T
                    for b in range(N_CORES)]).astype(np.float32)
    return out, res


def kernel(x, Wk, Wq, Wv):
    out, _ = _run(x, Wk, Wq, Wv)
    return out
